# revision 2
# baseline (speedup 1.0000x reference)
"""Trainium2 Bass kernel for per-pixel cosine-distance block.

x1: [B, C, h, w]  f32
x2: [B, S, C, h, w] f32
out: [B, S*h*w] f32  where out[b, s*h*w + p] = 1 - cos(x1[b,:,p], x2[b,s,:,p])
(cosine over the channel dim C, per pixel)

Sharding: data-parallel over B across 8 NeuronCores (4 batches per core).

The kernel is HBM-read bound (72 MiB/core at ~360-420 GB/s/core), so the
whole design serves keeping the input DMA stream gapless:

  * Loads are plain f32 on the two HWDGE rings (x2 on the sync ring, x1
    on the scalar ring). The previous SWDGE cast-load version was capped
    by SDMA engine 15, which runs ~20% slower under SWDGE (descriptor-
    ring port contention) and stretched every transfer; HWDGE has no
    descriptor rings, so all 16 engines run uniformly. The f32->bf16
    cast rides for free inside the elementwise ops that consume x2.

  * Normalization happens at the END, not up front: the per-tile work is
    prod = x1*x2 (raw dot terms) and sq2 = x2^2, reduced over C by
    one-hot matmuls into PSUM, and the epilogue computes
    dist = 1 - dot*rsqrt(ss1)*rsqrt(ss2). prod(s0) therefore needs only
    the raw x1/x2 bytes - the old qn = x1*rsqrt(sum x1^2) chain
    serialized ~45 us of startup before the first x2 buffer could free,
    stalling the load queue.

  * Elementwise work is spread over DVE, ACT and the (otherwise idle)
    GpSimd so no engine exceeds ~70% of the DMA cadence. PE does the
    partition reductions: s-tile s uses PE column group s%4
    (tile_position=(0,32g)); group g's rows live at PSUM partitions
    32g..32g+1 and its accumulation closes at s=g+4, so epilogues and
    stores spread through the batch. Output stores are SWDGE (gpsimd)
    and immediately follow their producer on the same queue.

  * The per-batch x1-norm prep (sq1 -> ss1 -> rsqrt -> PE-broadcast of
    -rsqrt(ss1) to all partitions) is emitted in spare slots of the
    previous batch, off every critical path.
"""

from contextlib import ExitStack

import numpy as np

import concourse.bass as bass
import concourse.tile as tile
from concourse import bacc, mybir
from concourse.bass_utils import run_bass_kernel_spmd

B, S, C, H, W = 32, 8, 512, 32, 32
HW = H * W  # 1024
N_CORES = 8
BL = B // N_CORES  # 4 batches per core
P = 128
NCH = C // P  # 4 chunks of the channel dim
HWH = HW // 2  # 512 (one PSUM bank of f32)
NG = 4  # PE column groups; s-tile s -> group s % NG
SPG = S // NG  # s values per group (2)
NPR = 3 * 32 + SPG  # 98: all four group regions in one partition span

FP32 = mybir.dt.float32
BF16 = mybir.dt.bfloat16

RSQRT = mybir.ActivationFunctionType.Abs_reciprocal_sqrt
SQUARE = mybir.ActivationFunctionType.Square
COPY = mybir.ActivationFunctionType.Copy

# engine split of the per-tile elementwise work (by s index)
PROD_ON_GP = (2, 6)  # prod = x1*x2 on GpSimd for these s, DVE otherwise
SQ2_ON_GP = (1, 5)  # sq2 = x2^2 on GpSimd for these s, ACT otherwise


def _emit(ctx: ExitStack, tc: tile.TileContext, x1, x2, out):
    nc = tc.nc

    # c = p*NCH + k -> partition p, chunk k: 16 KiB contiguous per partition
    x1r = x1.rearrange("b (p k) f -> b p k f", p=P)  # [BL, 128, NCH, HW]
    x2r = x2.rearrange("b s (p k) f -> b s p k f", p=P)  # [BL, S, 128, NCH, HW]
    # s = j*NG + g -> out rows of group g hold s in {g, g+4}
    outr = out.rearrange("b (j g) f -> b g j f", g=NG)  # [BL, NG, SPG, HW]

    singles = ctx.enter_context(tc.tile_pool(name="singles", bufs=1))
    x1_pool = ctx.enter_context(tc.tile_pool(name="x1", bufs=2))
    x2_pool = ctx.enter_context(tc.tile_pool(name="x2", bufs=5))
    sq1_pool = ctx.enter_context(tc.tile_pool(name="sq1", bufs=1))
    prod_pool = ctx.enter_context(tc.tile_pool(name="prod", bufs=3))
    sq2_pool = ctx.enter_context(tc.tile_pool(name="sq2", bufs=3))
    rr1_pool = ctx.enter_context(tc.tile_pool(name="rr1", bufs=1))
    repn_pool = ctx.enter_context(tc.tile_pool(name="repn", bufs=2))
    rr2_pool = ctx.enter_context(tc.tile_pool(name="rr2", bufs=2))
    t2_pool = ctx.enter_context(tc.tile_pool(name="t2", bufs=2))
    dist_pool = ctx.enter_context(tc.tile_pool(name="dist", bufs=2))
    # PSUM: 8 banks = dot acc (2) + ss2 acc (2) + ss1 (2) + rep (2)
    pdot_pool = ctx.enter_context(tc.tile_pool(name="pdot", bufs=1, space="PSUM"))
    pss2_pool = ctx.enter_context(tc.tile_pool(name="pss2", bufs=1, space="PSUM"))
    ss1_pool = ctx.enter_context(tc.tile_pool(name="ss1p", bufs=1, space="PSUM"))
    rep_pool = ctx.enter_context(tc.tile_pool(name="rep", bufs=1, space="PSUM"))

    # oh2[:, j, :] is a [P, 2] matrix, all-ones in column j: as lhsT it
    # deposits the partition-reduction of rhs into row j of the 2-row
    # group region (adding zero to the other row).
    oh2 = singles.tile([P, SPG, SPG], BF16)
    nc.vector.memset(oh2, 0.0)
    for r in range(SPG):
        nc.vector.memset(oh2[:, r, r : r + 1], 1.0)
    ones1 = singles.tile([P, 1], BF16)
    nc.vector.memset(ones1, 1.0)
    # [1, P] of -1: K=1 matmul with it as lhsT replicates (and negates) an
    # SBUF row across all 128 PSUM partitions.
    neg128 = singles.tile([1, P], FP32)
    nc.vector.memset(neg128, -1.0)

    # ---- per-batch x1-norm prep, staged over several s-slots ----------

    def prep_load(b):
        x1_t = x1_pool.tile([P, NCH, HW], FP32)
        nc.scalar.dma_start(x1_t[:], x1r[b])
        return x1_t

    def prep_sq1(x1_t):
        sq1 = sq1_pool.tile([P, NCH, HW], BF16)
        nc.vector.tensor_mul(sq1[:], x1_t[:], x1_t[:])
        return sq1

    def prep_ss1(sq1):
        ss1 = ss1_pool.tile([1, 2, HWH], FP32)
        for hh in range(2):
            for ic in range(NCH):
                nc.tensor.matmul(
                    ss1[:, hh, :],
                    ones1,
                    sq1[:, ic, hh * HWH : (hh + 1) * HWH],
                    start=(ic == 0),
                    stop=(ic == NCH - 1),
                )
        return ss1

    def prep_rr1(ss1):
        rr1 = rr1_pool.tile([1, 2, HWH], FP32)
        nc.scalar.activation(rr1[:], ss1[:], func=RSQRT)
        return rr1

    def prep_rep(rr1):
        # repn = -rsqrt(ss1) broadcast to all partitions (PE K=1 matmul),
        # then copied off PSUM so the banks free immediately.
        rep = rep_pool.tile([P, 2, HWH], FP32)
        for hh in range(2):
            nc.tensor.matmul(rep[:, hh, :], neg128, rr1[:, hh, :], start=True, stop=True)
        repn = repn_pool.tile([P, 2, HWH], FP32)
        nc.scalar.activation(repn[:], rep[:], func=COPY)
        return repn

    # ---- per-group epilogue ------------------------------------------

    def epilogue(b, g, pdot, pss2, repn, rr2_t, t2_t, dist_t, hh_list):
        # dist = 1 - dot*rsqrt(ss1)*rsqrt(ss2) for group g's two rows
        rows = slice(32 * g, 32 * g + SPG)
        nh = len(hh_list)
        h0 = hh_list[0]
        hsl = slice(h0, h0 + nh)
        nc.scalar.activation(rr2_t[rows, hsl, :], pss2[rows, hsl, :], func=RSQRT)
        nc.gpsimd.tensor_mul(t2_t[rows, hsl, :], rr2_t[rows, hsl, :], repn[rows, hsl, :])
        nc.vector.tensor_mul(t2_t[rows, hsl, :], pdot[rows, hsl, :], t2_t[rows, hsl, :])
        nc.gpsimd.tensor_scalar_add(dist_t[rows, hsl, :], t2_t[rows, hsl, :], 1.0)
        nc.gpsimd.dma_start(
            outr[b, g][:, h0 * HWH : (h0 + nh) * HWH], dist_t[rows, hsl, :]
        )

    # ---- main pipeline ------------------------------------------------

    x1_cur = prep_load(0)
    x1_nxt = None
    repn_cur = None
    repn_nxt = None
    sq1_t = None
    ss1_t = None
    rr1_t = None

    for b in range(BL):
        pdot = pdot_pool.tile([NPR, 2, HWH], FP32)  # 2 banks
        pss2 = pss2_pool.tile([NPR, 2, HWH], FP32)  # 2 banks
        rr2_t = rr2_pool.tile([NPR, 2, HWH], FP32)
        t2_t = t2_pool.tile([NPR, 2, HWH], FP32)
        dist_t = dist_pool.tile([NPR, 2, HWH], FP32)
        for s in range(S):
            g = s % NG
            j = s // NG
            last = b == BL - 1 and s == S - 1
            x2_t = x2_pool.tile([P, NCH, HW], FP32)
            if last:
                # split the final load so the tail drains per hw half
                for hh in range(2):
                    nc.sync.dma_start(
                        x2_t[:, :, hh * HWH : (hh + 1) * HWH],
                        x2r[b, s][:, :, hh * HWH : (hh + 1) * HWH],
                    )
            else:
                nc.sync.dma_start(x2_t[:], x2r[b, s])

            if last:
                prods = []
                for hh in range(2):
                    hsl = slice(hh * HWH, (hh + 1) * HWH)
                    prod = prod_pool.tile([P, NCH, HWH], BF16)
                    nc.vector.tensor_mul(prod[:], x1_cur[:, :, hsl], x2_t[:, :, hsl])
                    sq2 = sq2_pool.tile([P, NCH, HWH], BF16)
                    nc.scalar.activation(sq2[:], x2_t[:, :, hsl], func=SQUARE)
                    prods.append((prod, sq2))
            else:
                prod = prod_pool.tile([P, NCH, HW], BF16)
                if s in PROD_ON_GP:
                    nc.gpsimd.tensor_mul(prod[:], x1_cur[:], x2_t[:])
                else:
                    nc.vector.tensor_mul(prod[:], x1_cur[:], x2_t[:])
                sq2 = sq2_pool.tile([P, NCH, HW], BF16)
                if s in SQ2_ON_GP:
                    nc.gpsimd.tensor_mul(sq2[:], x2_t[:], x2_t[:])
                else:
                    nc.scalar.activation(sq2[:], x2_t[:], func=SQUARE)

            # group g accumulates row j at partitions 32g..32g+1
            rows = slice(32 * g, 32 * g + SPG)

            def mm(kind, hh):
                if last:
                    src = prods[hh][kind]
                    csl = slice(0, HWH)
                else:
                    src = prod if kind == 0 else sq2
                    csl = slice(hh * HWH, (hh + 1) * HWH)
                acc = pdot if kind == 0 else pss2
                for ic in range(NCH):
                    nc.tensor.matmul(
                        acc[rows, hh, :],
                        oh2[:, j, :],
                        src[:, ic, csl],
                        start=(j == 0 and ic == 0),
                        stop=(j == SPG - 1 and ic == NCH - 1),
                        tile_position=(0, 32 * g),
                    )

            if last:
                for hh in range(2):
                    mm(0, hh)
                    mm(1, hh)
                    epilogue(b, g, pdot, pss2, repn_cur, rr2_t, t2_t, dist_t, [hh])
            else:
                mm(0, 0)
                mm(0, 1)
                mm(1, 0)
                mm(1, 1)
                if s >= NG:
                    epilogue(
                        b, s - NG, pdot, pss2, repn_cur, rr2_t, t2_t, dist_t, [0, 1]
                    )

            # staged prep: batch 0 preps itself in its first slots; later
            # batches are prepped during the previous batch.
            if b == 0:
                if s == 0:
                    sq1_t = prep_sq1(x1_cur)
                elif s == 1:
                    ss1_t = prep_ss1(sq1_t)
                elif s == 2:
                    rr1_t = prep_rr1(ss1_t)
                elif s == 3:
                    repn_cur = prep_rep(rr1_t)
            if b + 1 < BL:
                if s == 0:
                    x1_nxt = prep_load(b + 1)
                elif s == 2:
                    sq1_t = prep_sq1(x1_nxt)
                elif s == 3:
                    ss1_t = prep_ss1(sq1_t)
                elif s == 4:
                    rr1_t = prep_rr1(ss1_t)
                elif s == 5:
                    repn_nxt = prep_rep(rr1_t)

        x1_cur = x1_nxt
        repn_cur = repn_nxt


def _build():
    # Bacc (not plain Bass): its compile pipeline legalizes TRN2's
    # one-sync-wait-per-instruction limit (generate_event_semaphores).
    nc = bacc.Bacc("TRN2")
    x1 = nc.dram_tensor("x1", [BL, C, HW], FP32, kind="ExternalInput")
    x2 = nc.dram_tensor("x2", [BL, S, C, HW], FP32, kind="ExternalInput")
    out = nc.dram_tensor("out", [BL, S, HW], FP32, kind="ExternalOutput")
    with tile.TileContext(nc) as tc:
        with ExitStack() as ctx:
            _emit(ctx, tc, x1[:], x2[:], out[:])
    nc.finalize()
    return nc


_NC = None

# test-harness knobs (the grading harness never touches these)
TRACE = False
TRACE_DIR = None
LAST_RESULTS = None


def _get_nc():
    global _NC
    if _NC is None:
        _NC = _build()
    return _NC


def kernel(x1: np.ndarray, x2: np.ndarray) -> np.ndarray:
    global LAST_RESULTS
    x1 = np.ascontiguousarray(x1, dtype=np.float32).reshape(B, C, HW)
    x2 = np.ascontiguousarray(x2, dtype=np.float32).reshape(B, S, C, HW)
    nc = _get_nc()
    in_maps = [
        {"x1": x1[c * BL : (c + 1) * BL], "x2": x2[c * BL : (c + 1) * BL]}
        for c in range(N_CORES)
    ]
    res = run_bass_kernel_spmd(
        nc, in_maps, list(range(N_CORES)), trace=TRACE, tmpdir=TRACE_DIR
    )
    LAST_RESULTS = res
    outs = [res.results[c]["out"].reshape(BL, S * HW) for c in range(N_CORES)]
    return np.concatenate(outs, axis=0)
